# revision 26
# baseline (speedup 1.0000x reference)
"""Trainium2 Bass kernel for nn_Map_79748952752358 (dense_cnn).

Pipeline per sample batch: LSTM encoder (40 steps) -> e2d projection ->
big linear (lin1: 256 -> 262144) -> per-sample dynamic 1x1 conv over
feature [1024, 32x32] -> BN(eval) -> channel-max -> clip.

Sharding: 8-way over the R=256 conv output channels (32 per core). Every
core runs the full LSTM + e2d (replicated), computes its 32-row slice of
the dynamic filters (lin1 tensor-sharded over rows), convolves against
the full feature tensor, and emits a per-core partial channel-max
[16, 1024].  The host combines partials with np.maximum, applies the
BN-shift floor T0 = max_r(t_r) and the final clip.

Key math folds (exact, done on host):
  - BN scale s=gamma/sqrt(var+eps) > 0 folded into lin1 weights/bias
    (LeakyRelu and Relu are positively homogeneous).
  - relu(x)+t maxed over r == max(max_r(x+t), max_r(t)); the +t is
    injected into the conv PSUM via a rank-1 matmul, max_r(t) on host.
"""

import os
import numpy as np
import ml_dtypes

import concourse.bass as bass
import concourse.mybir as mybir
from concourse import tile
from concourse.tile import ScopedClock
from concourse.alu_op_type import AluOpType
from concourse.bass_utils import run_bass_kernel_spmd

BF16 = ml_dtypes.bfloat16

B, S, V, E, HID = 16, 40, 1004, 256, 256
C, R, HW2 = 1024, 256, 1024
BN_EPS = 1e-5
N_CORES = 8
RS = R // N_CORES  # 32 r-rows per core
P = 128

AFT = mybir.ActivationFunctionType
AX = mybir.AxisListType


# ---------------------------------------------------------------------------
# Tile tail-drain patch: this walrus build accepts fewer sem waits per
# TPB_CTRL instruction than Tile's exit drain accumulates; split them into
# single-wait SP nops.
_drain_patched = False


def _patch_tile_drain():
    global _drain_patched
    if _drain_patched:
        return
    _drain_patched = True

    def _patched(self, tick_clock, wait_clock):
        nc = self.nc
        probe = nc.sync.nop(nofuse=True, hint="drain_wait_split")
        wait_clock.add_sem_waits(
            probe.ins, ScopedClock({None: tick_clock.global_clock})
        )
        si = probe.ins.sync_info
        waits = list(si.on_wait or []) if si is not None else []
        if len(waits) > 1:
            si.on_wait = waits[:1]
            for w in waits[1:]:
                n = nc.sync.nop(nofuse=True, hint="drain_wait_split")
                nsi = n.ins.sync_info
                if nsi is None:
                    import bass_rust

                    n.ins.sync_info = bass_rust.SyncInfo(on_wait=[w], on_update=[])
                else:
                    nsi.on_wait = [w]
        nc.sync.drain()
        nc.all_engine_barrier()
        assert self.sems is not None
        popped = nc._tile_sem_poison_stack.pop()
        assert popped is self._sem_poison
        nc.clear_and_free_semaphores(list(self.sems.allocated().values()))
        nc.all_engine_barrier()

    tile.TileContext._drain_and_barrier = _patched


_ws_counter = [0]


def _split_excess_waits(nc, limit=1):
    """Walrus on this image rejects instructions with more than ~2 sem waits.
    Move excess waits onto same-engine EventSemaphore carriers inserted just
    before the offending instruction (same per-engine stream order, identical
    blocking semantics)."""
    import bass_rust

    for fn in nc.m.functions:
        for bb in fn.blocks:
            out = []
            for inst in bb.instructions:
                si = inst.sync_info
                waits = list(si.on_wait or []) if si is not None else []
                if len(waits) > limit:
                    for w in waits[:-limit]:
                        _ws_counter[0] += 1
                        carrier = mybir.InstEventSemaphore(
                            name=f"I-waitsplit-{_ws_counter[0]}",
                            opcode="EventSemaphore",
                            engine=inst.engine,
                            sync_info=bass_rust.SyncInfo(
                                on_wait=[w], on_update=[]),
                        )
                        out.append(carrier)
                    si.on_wait = waits[-limit:]
                out.append(inst)
            bb.instructions = out


# ---------------------------------------------------------------------------
def _build_program(slots):
    """Build the SPMD Bass program. `slots[b]` = length[b]-1, the LSTM step
    whose hidden state is each sample's final state (compile-time constants).
    """
    _patch_tile_drain()
    nc = bass.Bass("TRN2", target_bir_lowering=False, debug=False,
                   num_devices=N_CORES)
    dt = mybir.dt
    f32, bf16 = dt.float32, dt.bfloat16

    def din(name, shape, d=bf16):
        return nc.dram_tensor(name, shape, d, kind="ExternalInput").ap()

    feat_d = din("feat", [B, P, 8, HW2])          # (b, c_in, c-tile, hw) bf16
    # one packed bf16 constant block: embT | wihT | whhT | e2dT | eye |
    # b1 | trow | delta | ones32   (the 32-partition blocks are zero-padded)
    PK = [2 * S * B, 16 * P, 16 * P, 4 * P, P, 8 * P, 8 * RS, RS * B, P]
    pack_d = din("cpack", [P, sum(PK)])
    biasf_d = din("biasf", [P, 10], f32)          # biasg(8) | e2db(2)
    w1_d = din("w1T", [P, 8 * RS * 2 * P])        # tiles (ct, r, kh)

    out_d = nc.dram_tensor("part_out", [P, B * 8], f32, kind="ExternalOutput").ap()

    with tile.TileContext(nc) as tc:
        with (
            tc.tile_pool(name="const", bufs=1) as cpool,
            tc.tile_pool(name="xg", bufs=1) as xgpool,
            tc.tile_pool(name="hist", bufs=1) as hpool,
            tc.tile_pool(name="gs", bufs=2) as gspool,
            tc.tile_pool(name="cell", bufs=1) as cellpool,
            tc.tile_pool(name="tmp", bufs=4) as tmppool,
            tc.tile_pool(name="w1c", bufs=2) as w1pool,
            tc.tile_pool(name="f1", bufs=1) as f1pool,
            tc.tile_pool(name="feat", bufs=7) as fpool,
            tc.tile_pool(name="vout", bufs=1) as vpool,
        ):
            pack = cpool.tile([P, sum(PK)], bf16, tag="cpack")
            nc.sync.dma_start(out=pack[:], in_=pack_d)
            biasf = cpool.tile([P, 10], f32, tag="biasf")
            nc.sync.dma_start(out=biasf[:], in_=biasf_d)

            off = np.cumsum([0] + PK)
            embT = pack[:, off[0]:off[1]]
            wih = pack[:, off[1]:off[2]]
            whh = pack[:, off[2]:off[3]]
            e2dT = pack[:, off[3]:off[4]]
            eye = pack[:, off[4]:off[5]]
            b1 = pack[0:RS, off[5]:off[6]]
            trow = pack[0:RS, off[6]:off[7]]
            delta = pack[0:RS, off[7]:off[8]]
            ones32 = pack[0:RS, off[8]:off[9]]
            biasg = biasf[:, 0:8]
            e2db = biasf[:, 8:10]

            # ---- Stage A: xg = w_ih @ x_t for all steps (+ gate bias) ----
            # out tiles: xg_s[p, m*640 + t*16 + b]  (bf16)
            xg_s = xgpool.tile([P, 8 * S * B], bf16)
            NCH = 320  # psum N-chunk: 20 steps x 16
            with tc.tile_pool(name="xpsum", bufs=2, space="PSUM") as xpsum:
                for m in range(8):
                    for n in range(2):
                        ps = xpsum.tile([P, NCH], f32, tag="xg")
                        for ke in range(2):
                            nc.tensor.matmul(
                                ps[:],
                                lhsT=wih[:, (ke * 8 + m) * P:(ke * 8 + m + 1) * P],
                                rhs=embT[:, ke * S * B + n * NCH: ke * S * B + (n + 1) * NCH],
                                start=(ke == 0), stop=(ke == 1),
                            )
                        nc.scalar.activation(
                            out=xg_s[:, m * S * B + n * NCH: m * S * B + (n + 1) * NCH],
                            in_=ps[:], func=AFT.Identity, bias=biasg[:, m:m + 1],
                        )

            # ---- Stage B: LSTM recurrence (layout: gate-dim on partitions) --
            hist = hpool.tile([P, S * 2 * B], bf16)   # (t, kh, b)
            c_s = cellpool.tile([P, 2 * B], f32)      # (kh, b)
            xg_r = xg_s[:].rearrange("p (m t b) -> p m t b", m=8, t=S)
            lstm_psum = tc.tile_pool(name="gpsum", bufs=2, space="PSUM")
            gpsum = lstm_psum.__enter__()
            for t in range(S):
                gp = gpsum.tile([P, P], f32, tag="gates")
                nc.tensor.matmul(gp[:], lhsT=eye[:], rhs=xg_r[:, :, t, :],
                                 start=True, stop=(t == 0))
                if t > 0:
                    for m in range(8):
                        for kh in range(2):
                            nc.tensor.matmul(
                                gp[:, m * B:(m + 1) * B],
                                lhsT=whh[:, (kh * 8 + m) * P:(kh * 8 + m + 1) * P],
                                rhs=hist[:, (t - 1) * 2 * B + kh * B:
                                         (t - 1) * 2 * B + (kh + 1) * B],
                                start=False, stop=(m == 7 and kh == 1),
                                skip_group_check=True,
                            )
                gs = gspool.tile([P, P], f32, tag="gs")
                # cols (m,b): i=0:32, f=32:64, g=64:96, o=96:128
                # one sigmoid for all gates; tanh(g) = 2*sig(2g)-1 (g-rows
                # pre-scaled by 2 on host).  fp32: 2*sig-1 near sig=0.5
                # amplifies rounding into absolute tanh error otherwise.
                nc.scalar.activation(out=gs[:], in_=gp[:], func=AFT.Sigmoid)
                tg = tmppool.tile([P, 2 * B], f32, tag="tg")
                nc.vector.tensor_scalar(tg[:], gs[:, 64:96], 2.0, -1.0,
                                        AluOpType.mult, AluOpType.add)
                t1 = tmppool.tile([P, 2 * B], f32, tag="t1")
                nc.vector.tensor_tensor(t1[:], gs[:, 0:32], tg[:],
                                        AluOpType.mult)
                if t == 0:
                    nc.vector.tensor_copy(c_s[:], t1[:])
                else:
                    t2 = tmppool.tile([P, 2 * B], f32, tag="t2")
                    nc.vector.tensor_tensor(t2[:], gs[:, 32:64], c_s[:],
                                            AluOpType.mult)
                    nc.vector.tensor_tensor(c_s[:], t1[:], t2[:], AluOpType.add)
                th = tmppool.tile([P, 2 * B], bf16, tag="th")
                nc.scalar.activation(out=th[:], in_=c_s[:], func=AFT.Tanh)
                nc.vector.tensor_tensor(
                    hist[:, t * 2 * B:(t + 1) * 2 * B],
                    gs[:, 96:128], th[:], AluOpType.mult)

            # ---- capture final h per sample (slots known at build time) ----
            h_fin = cellpool.tile([P, 2 * B], bf16, tag="hfin")  # (kh, b)
            hf_r = h_fin[:].rearrange("p (k b) -> p b k", k=2)
            for b in range(B):
                src = hist[:, slots[b] * 2 * B:(slots[b] + 1) * 2 * B]
                nc.vector.tensor_copy(
                    hf_r[:, b], src.rearrange("p (k b) -> p b k", k=2)[:, b])

            # ---- e2d projection: instrT = tanh(e2d_w @ h + b) -------------
            instrT = cellpool.tile([P, 2 * B], bf16, tag="instrT")  # (kh, b)
            for m in range(2):
                pe2 = gpsum.tile([P, B], f32, tag="e2d")
                for kh in range(2):
                    nc.tensor.matmul(
                        pe2[:],
                        lhsT=e2dT[:, (kh * 2 + m) * P:(kh * 2 + m + 1) * P],
                        rhs=h_fin[:, kh * B:(kh + 1) * B],
                        start=(kh == 0), stop=(kh == 1),
                    )
                nc.scalar.activation(out=instrT[:, m * B:(m + 1) * B],
                                     in_=pe2[:], func=AFT.Tanh,
                                     bias=e2db[:, m:m + 1])
            lstm_psum.__exit__(None, None, None)

            # ---- lin1 (r-slice): f1T[c, (ct,b,r)] = Lrelu(W_ct_r @ instr + b1)
            f1_sb = f1pool.tile([P, 8 * B * RS], bf16)
            CW = RS * 2 * P  # w1 chunk cols per ct
            lin1_psum = tc.tile_pool(name="lpsum", bufs=4, space="PSUM")
            lpsum = lin1_psum.__enter__()
            for ct in range(8):
                wch = w1pool.tile([P, CW], bf16, tag="w1c")
                nc.sync.dma_start(out=wch[:], in_=w1_d[:, ct * CW:(ct + 1) * CW])
                pb = lpsum.tile([P, RS * B], f32, tag="lin1")
                nc.tensor.matmul(pb[:], lhsT=b1[:, ct * P:(ct + 1) * P],
                                 rhs=delta[:], start=True, stop=False,
                                 skip_group_check=True)
                for r in range(RS):
                    for kh in range(2):
                        nc.tensor.matmul(
                            pb[:, r * B:(r + 1) * B],
                            lhsT=wch[:, (r * 2 + kh) * P:(r * 2 + kh + 1) * P],
                            rhs=instrT[:, kh * B:(kh + 1) * B],
                            start=False, stop=(r == RS - 1 and kh == 1),
                            skip_group_check=True,
                        )
                out_ap = (f1_sb[:, ct * B * RS:(ct + 1) * B * RS]
                          .rearrange("p (b r) -> p r b", b=B))
                nc.scalar.activation(out=out_ap, in_=pb[:], func=AFT.Lrelu,
                                     alpha=0.01)
            lin1_psum.__exit__(None, None, None)

            # ---- conv per sample + fused BN-shift + channel max ----------
            vout = vpool.tile([P, B * 8], f32)  # (b, m)
            conv_psum = tc.tile_pool(name="cpsum", bufs=4, space="PSUM")
            cpsum = conv_psum.__enter__()
            for b in range(B):
                fb = fpool.tile([P, 8 * HW2], bf16, tag="feat")
                nc.sync.dma_start(
                    out=fb[:].rearrange("p (kc hw) -> p kc hw", kc=8),
                    in_=feat_d[b])
                pc = cpsum.tile([P, 8 * RS], f32, tag="conv")
                nc.tensor.matmul(pc[:], lhsT=ones32[:], rhs=trow[:],
                                 start=True, stop=False, skip_group_check=True)
                for m in range(8):
                    for kc in range(8):
                        nc.tensor.matmul(
                            pc[:, m * RS:(m + 1) * RS],
                            lhsT=fb[:, kc * HW2 + m * P: kc * HW2 + (m + 1) * P],
                            rhs=f1_sb[:, kc * B * RS + b * RS:
                                      kc * B * RS + (b + 1) * RS],
                            start=False, stop=(m == 7 and kc == 7),
                            skip_group_check=True,
                        )
                nc.vector.tensor_reduce(
                    out=vout[:, b * 8:(b + 1) * 8],
                    in_=pc[:].rearrange("p (m r) -> p m r", m=8),
                    axis=AX.X, op=AluOpType.max)
            conv_psum.__exit__(None, None, None)

            # contiguous store; host un-permutes (p,(b,m)) -> (b, m*128+p)
            nc.sync.dma_start(out=out_d, in_=vout[:])

    _split_excess_waits(nc)
    return nc


# ---------------------------------------------------------------------------
def _prep_inputs(feature, instruction_idx, instruction_length, emb_table,
                 w_ih, w_hh, b_ih, b_hh, e2d_w, e2d_b,
                 lin1_w, lin1_b, bn_gamma, bn_beta, bn_mean, bn_var):
    """Host-side layout/dtype prep. Returns (in_maps, slots, T0)."""
    f32 = np.float32

    def to_bf(x):
        return np.ascontiguousarray(x.astype(BF16))

    feature = np.asarray(feature, f32)
    emb_table = np.asarray(emb_table, f32)
    idx = np.asarray(instruction_idx)
    lengths = np.asarray(instruction_length).astype(np.int64)
    slots = [int(max(l, 1) - 1) for l in lengths]

    # feature (b, c_in, kc, hw): per-partition data contiguous (16KB) so the
    # DMA uses 16KB descriptors instead of 2KB
    feat = to_bf(feature.reshape(B, 8, P, HW2).transpose(0, 2, 1, 3))

    # embeds transposed: [p, (ke, t*b)]
    emb = emb_table[idx]                       # [B, S, E]
    embT = emb.transpose(2, 1, 0).reshape(2, P, S * B)
    embT = to_bf(embT.transpose(1, 0, 2).reshape(P, 2 * S * B))

    def wtiles(w, kt, mt):
        # w: [out, in] -> lhsT tiles arr[p, (k, m, col)] with lhsT=w.T tile
        wt = np.asarray(w, f32).T  # [in, out]
        a = wt.reshape(kt, P, mt, P).transpose(1, 0, 2, 3)
        return to_bf(a.reshape(P, kt * mt * P))

    # tanh(g) computed as 2*sigmoid(2g)-1: scale the g-gate rows (512:768)
    # by 2 so one big sigmoid covers all four gates.
    gsc = np.ones((4 * HID, 1), f32)
    gsc[2 * HID:3 * HID] = 2.0
    wihT = wtiles(np.asarray(w_ih, f32) * gsc, 2, 8)
    whhT = wtiles(np.asarray(w_hh, f32) * gsc, 2, 8)
    e2dT = wtiles(e2d_w, 2, 2)

    bg = ((np.asarray(b_ih, f32) + np.asarray(b_hh, f32)) * gsc[:, 0]) \
        .reshape(8, P).T.copy()
    e2db = np.asarray(e2d_b, f32).reshape(2, P).T.copy()

    s = np.asarray(bn_gamma, f32) / np.sqrt(np.asarray(bn_var, f32) + BN_EPS)
    tsh = np.asarray(bn_beta, f32) - np.asarray(bn_mean, f32) * s
    T0 = float(tsh.max())

    w1s = np.asarray(lin1_w, f32).reshape(R, C, HID) * s[:, None, None]
    b1s = np.asarray(lin1_b, f32).reshape(R, C) * s[:, None]

    delta = np.repeat(np.eye(RS, dtype=f32), B, axis=1)  # [32, 512]
    eye = np.eye(P, dtype=f32)
    ones32 = np.ones((RS, P), f32)

    def pad128(a):
        out = np.zeros((P, a.shape[1]), f32)
        out[:a.shape[0]] = a
        return out

    biasf = np.concatenate([bg, e2db], axis=1).astype(f32)  # [128, 10]
    biasf = np.ascontiguousarray(biasf)

    in_maps = []
    for k in range(N_CORES):
        rsl = slice(k * RS, (k + 1) * RS)
        wsl = w1s[rsl]                          # [32, 1024, 256] (r, c, h)
        # tiles (ct, r, kh): arr[p, ...] = w.T[kh*128+p, r, ct*128+col]
        ws = wsl.transpose(2, 1, 0)             # [h, c, r]
        a = (ws.reshape(2, P, 8, P, RS)         # [kh, p, ct, col, r]
             .transpose(1, 2, 4, 0, 3)          # [p, ct, r, kh, col]
             .reshape(P, 8 * RS * 2 * P))
        b1c = b1s[rsl].reshape(RS, 8, P).reshape(RS, 8 * P)  # (r, (ct, c))
        tr = np.zeros((RS, 8 * RS), f32)
        tr[0] = np.tile(tsh[rsl], 8)
        cpack = np.concatenate(
            [embT.astype(f32), wihT.astype(f32), whhT.astype(f32),
             e2dT.astype(f32), eye, pad128(b1c), pad128(tr), pad128(delta),
             pad128(ones32)], axis=1)
        in_maps.append(dict(feat=feat, cpack=to_bf(cpack), biasf=biasf,
                            w1T=to_bf(a)))
    return in_maps, slots, T0


_cache = {}


def _run(inputs, trace=False):
    (in_maps, slots, T0) = _prep_inputs(
        inputs["feature"], inputs["instruction_idx"],
        inputs["instruction_length"], inputs["emb_table"],
        inputs["w_ih"], inputs["w_hh"], inputs["b_ih"], inputs["b_hh"],
        inputs["e2d_w"], inputs["e2d_b"], inputs["lin1_w"], inputs["lin1_b"],
        inputs["bn_gamma"], inputs["bn_beta"], inputs["bn_mean"],
        inputs["bn_var"])

    key = tuple(slots)
    if key not in _cache:
        _cache[key] = _build_program(slots)
    nc = _cache[key]

    kw = {}
    if trace:
        kw = dict(trace=True, trace_cores=list(range(N_CORES)))
    res = run_bass_kernel_spmd(nc, in_maps, list(range(N_CORES)), **kw)
    parts = np.stack([np.asarray(res.results[i]["part_out"], np.float32)
                      for i in range(N_CORES)])        # [8, 128(p), 128(b,m)]
    single = parts.max(axis=0)                          # [p, (b, m)]
    # (p, (b, m)) -> (b, m*128+p)
    single = single.reshape(P, B, 8).transpose(1, 2, 0).reshape(B, HW2)
    single = np.maximum(single, T0)
    out = np.clip(single, 0.0, 1.0).reshape(B, 32, 32).astype(np.float32)
    return out, res


def kernel(**inputs) -> np.ndarray:
    out, _ = _run(inputs, trace=False)
    return out


def kernel_traced(**inputs):
    out, res = _run(inputs, trace=True)
    return out, res


# revision 31
# speedup vs baseline: 1.0168x; 1.0168x over previous
"""Trainium2 Bass kernel for nn_Map_79748952752358 (dense_cnn).

Pipeline per sample batch: LSTM encoder (40 steps) -> e2d projection ->
big linear (lin1: 256 -> 262144) -> per-sample dynamic 1x1 conv over
feature [1024, 32x32] -> BN(eval) -> channel-max -> clip.

Sharding: 8-way over the R=256 conv output channels (32 per core). Every
core runs the full LSTM + e2d (replicated), computes its 32-row slice of
the dynamic filters (lin1 tensor-sharded over rows), convolves against
the full feature tensor, and emits a per-core partial channel-max
[16, 1024].  The host combines partials with np.maximum, applies the
BN-shift floor T0 = max_r(t_r) and the final clip.

Key math folds (exact, done on host):
  - BN scale s=gamma/sqrt(var+eps) > 0 folded into lin1 weights/bias
    (LeakyRelu and Relu are positively homogeneous).
  - relu(x)+t maxed over r == max(max_r(x+t), max_r(t)); the +t is
    injected into the conv PSUM via a rank-1 matmul, max_r(t) on host.
"""

import os
import numpy as np
import ml_dtypes

import concourse.bass as bass
import concourse.mybir as mybir
from concourse import tile
from concourse.tile import ScopedClock
from concourse.alu_op_type import AluOpType
from concourse.bass_utils import run_bass_kernel_spmd

BF16 = ml_dtypes.bfloat16

B, S, V, E, HID = 16, 40, 1004, 256, 256
C, R, HW2 = 1024, 256, 1024
BN_EPS = 1e-5
N_CORES = 8
RS = R // N_CORES  # 32 r-rows per core
P = 128

AFT = mybir.ActivationFunctionType
AX = mybir.AxisListType


# ---------------------------------------------------------------------------
# Tile tail-drain patch: this walrus build accepts fewer sem waits per
# TPB_CTRL instruction than Tile's exit drain accumulates; split them into
# single-wait SP nops.
_drain_patched = False


def _patch_tile_drain():
    global _drain_patched
    if _drain_patched:
        return
    _drain_patched = True

    def _patched(self, tick_clock, wait_clock):
        nc = self.nc
        probe = nc.sync.nop(nofuse=True, hint="drain_wait_split")
        wait_clock.add_sem_waits(
            probe.ins, ScopedClock({None: tick_clock.global_clock})
        )
        si = probe.ins.sync_info
        waits = list(si.on_wait or []) if si is not None else []
        if len(waits) > 1:
            si.on_wait = waits[:1]
            for w in waits[1:]:
                n = nc.sync.nop(nofuse=True, hint="drain_wait_split")
                nsi = n.ins.sync_info
                if nsi is None:
                    import bass_rust

                    n.ins.sync_info = bass_rust.SyncInfo(on_wait=[w], on_update=[])
                else:
                    nsi.on_wait = [w]
        nc.sync.drain()
        nc.all_engine_barrier()
        assert self.sems is not None
        popped = nc._tile_sem_poison_stack.pop()
        assert popped is self._sem_poison
        nc.clear_and_free_semaphores(list(self.sems.allocated().values()))
        nc.all_engine_barrier()

    tile.TileContext._drain_and_barrier = _patched


_ws_counter = [0]


def _split_excess_waits(nc, limit=1):
    """Walrus on this image rejects instructions with more than ~2 sem waits.
    Move excess waits onto same-engine EventSemaphore carriers inserted just
    before the offending instruction (same per-engine stream order, identical
    blocking semantics)."""
    import bass_rust

    for fn in nc.m.functions:
        for bb in fn.blocks:
            out = []
            for inst in bb.instructions:
                si = inst.sync_info
                waits = list(si.on_wait or []) if si is not None else []
                if len(waits) > limit:
                    for w in waits[:-limit]:
                        _ws_counter[0] += 1
                        carrier = mybir.InstEventSemaphore(
                            name=f"I-waitsplit-{_ws_counter[0]}",
                            opcode="EventSemaphore",
                            engine=inst.engine,
                            sync_info=bass_rust.SyncInfo(
                                on_wait=[w], on_update=[]),
                        )
                        out.append(carrier)
                    si.on_wait = waits[-limit:]
                out.append(inst)
            bb.instructions = out


# ---------------------------------------------------------------------------
def _build_program(slots):
    """Build the SPMD Bass program. `slots[b]` = length[b]-1, the LSTM step
    whose hidden state is each sample's final state (compile-time constants).
    """
    _patch_tile_drain()
    nc = bass.Bass("TRN2", target_bir_lowering=False, debug=False,
                   num_devices=N_CORES)
    dt = mybir.dt
    f32, bf16 = dt.float32, dt.bfloat16

    def din(name, shape, d=bf16):
        return nc.dram_tensor(name, shape, d, kind="ExternalInput").ap()

    feat_d = din("feat", [B, P, 8, HW2])          # (b, c_in, c-tile, hw) bf16
    # one packed bf16 constant block: embT | wihT | whhT | e2dT | eye |
    # b1 | trow | delta | ones32   (the 32-partition blocks are zero-padded)
    PK = [2 * S * B, 16 * P, 16 * P, 4 * P, P, 8 * P, 8 * RS, RS * B, P]
    pack_d = din("cpack", [P, sum(PK)])
    biasf_d = din("biasf", [P, 10], f32)          # biasg(8) | e2db(2)
    w1_d = din("w1T", [P, 8 * RS * 2 * P])        # tiles (ct, r, kh)

    out_d = nc.dram_tensor("part_out", [P, B * 8], f32, kind="ExternalOutput").ap()

    with tile.TileContext(nc) as tc:
        with (
            tc.tile_pool(name="const", bufs=1) as cpool,
            tc.tile_pool(name="xg", bufs=1) as xgpool,
            tc.tile_pool(name="hist", bufs=1) as hpool,
            tc.tile_pool(name="gs", bufs=2) as gspool,
            tc.tile_pool(name="cell", bufs=1) as cellpool,
            tc.tile_pool(name="tmp", bufs=4) as tmppool,
            tc.tile_pool(name="w1c", bufs=2) as w1pool,
            tc.tile_pool(name="f1", bufs=1) as f1pool,
            tc.tile_pool(name="feat", bufs=7) as fpool,
            tc.tile_pool(name="vout", bufs=1) as vpool,
        ):
            pack = cpool.tile([P, sum(PK)], bf16, tag="cpack")
            nc.sync.dma_start(out=pack[:], in_=pack_d)
            biasf = cpool.tile([P, 10], f32, tag="biasf")
            nc.sync.dma_start(out=biasf[:], in_=biasf_d)

            off = np.cumsum([0] + PK)
            embT = pack[:, off[0]:off[1]]
            wih = pack[:, off[1]:off[2]]
            whh = pack[:, off[2]:off[3]]
            e2dT = pack[:, off[3]:off[4]]
            eye = pack[:, off[4]:off[5]]
            b1 = pack[0:RS, off[5]:off[6]]
            trow = pack[0:RS, off[6]:off[7]]
            delta = pack[0:RS, off[7]:off[8]]
            ones32 = pack[0:RS, off[8]:off[9]]
            biasg = biasf[:, 0:8]
            e2db = biasf[:, 8:10]

            # ---- Stage A: xg = w_ih @ x_t for all steps (+ gate bias) ----
            # out tiles: xg_s[p, m*640 + t*16 + b]  (bf16)
            xg_s = xgpool.tile([P, 8 * S * B], bf16)
            NCH = 320  # psum N-chunk: 20 steps x 16
            with tc.tile_pool(name="xpsum", bufs=2, space="PSUM") as xpsum:
                for m in range(8):
                    for n in range(2):
                        ps = xpsum.tile([P, NCH], f32, tag="xg")
                        for ke in range(2):
                            nc.tensor.matmul(
                                ps[:],
                                lhsT=wih[:, (ke * 8 + m) * P:(ke * 8 + m + 1) * P],
                                rhs=embT[:, ke * S * B + n * NCH: ke * S * B + (n + 1) * NCH],
                                start=(ke == 0), stop=(ke == 1),
                            )
                        nc.scalar.activation(
                            out=xg_s[:, m * S * B + n * NCH: m * S * B + (n + 1) * NCH],
                            in_=ps[:], func=AFT.Identity, bias=biasg[:, m:m + 1],
                        )

            # ---- Stage B: LSTM recurrence (layout: gate-dim on partitions) --
            # Two independent batch halves (b 0:8 / 8:16): their per-step
            # chains interleave across PE/ACT/DVE, roughly halving the
            # recurrence's serial-latency cost.
            HB = B // 2
            hists = [hpool.tile([P, S * 2 * HB], bf16, tag=f"hist{h}",
                                name=f"hist{h}")
                     for h in range(2)]          # (t, kh, b8)
            cs = [cellpool.tile([P, 2 * HB], f32, tag=f"c{h}", name=f"c{h}")
                  for h in range(2)]
            xg_r = xg_s[:].rearrange("p (m t b) -> p m t b", m=8, t=S)
            lstm_psum = tc.tile_pool(name="gpsum", bufs=2, space="PSUM")
            gpsum = lstm_psum.__enter__()
            for t in range(S):
                for h in range(2):
                    hist, c_s = hists[h], cs[h]
                    bs = slice(h * HB, (h + 1) * HB)
                    gp = gpsum.tile([P, 8 * HB], f32, tag=f"gates{h}")
                    nc.tensor.matmul(gp[:], lhsT=eye[:], rhs=xg_r[:, :, t, bs],
                                     start=True, stop=(t == 0))
                    if t > 0:
                        for m in range(8):
                            for kh in range(2):
                                nc.tensor.matmul(
                                    gp[:, m * HB:(m + 1) * HB],
                                    lhsT=whh[:, (kh * 8 + m) * P:
                                             (kh * 8 + m + 1) * P],
                                    rhs=hist[:, (t - 1) * 2 * HB + kh * HB:
                                             (t - 1) * 2 * HB + (kh + 1) * HB],
                                    start=False, stop=(m == 7 and kh == 1),
                                    skip_group_check=True,
                                )
                    gs = gspool.tile([P, 8 * HB], f32, tag=f"gs{h}")
                    # cols (m,b8): i=0:16, f=16:32, g=32:48, o=48:64
                    # one sigmoid for all gates; tanh(g)=2*sig(2g)-1 (g-rows
                    # pre-scaled by 2 on host; fp32 to avoid rounding blowup)
                    nc.scalar.activation(out=gs[:], in_=gp[:], func=AFT.Sigmoid)
                    tg = tmppool.tile([P, 2 * HB], f32, tag=f"tg{h}")
                    nc.vector.tensor_scalar(tg[:], gs[:, 32:48], 2.0, -1.0,
                                            AluOpType.mult, AluOpType.add)
                    t1 = tmppool.tile([P, 2 * HB], f32, tag=f"t1{h}")
                    nc.vector.tensor_tensor(t1[:], gs[:, 0:16], tg[:],
                                            AluOpType.mult)
                    if t == 0:
                        nc.vector.tensor_copy(c_s[:], t1[:])
                    else:
                        t2 = tmppool.tile([P, 2 * HB], f32, tag=f"t2{h}")
                        nc.vector.tensor_tensor(t2[:], gs[:, 16:32], c_s[:],
                                                AluOpType.mult)
                        nc.vector.tensor_tensor(c_s[:], t1[:], t2[:],
                                                AluOpType.add)
                    th = tmppool.tile([P, 2 * HB], bf16, tag=f"th{h}")
                    nc.scalar.activation(out=th[:], in_=c_s[:], func=AFT.Tanh)
                    nc.vector.tensor_tensor(
                        hist[:, t * 2 * HB:(t + 1) * 2 * HB],
                        gs[:, 48:64], th[:], AluOpType.mult)

            # ---- capture final h per sample (slots known at build time) ----
            h_fin = cellpool.tile([P, 2 * B], bf16, tag="hfin")  # (kh, b)
            hf_r = h_fin[:].rearrange("p (k b) -> p b k", k=2)
            for b in range(B):
                h = b // HB
                src = hists[h][:, slots[b] * 2 * HB:(slots[b] + 1) * 2 * HB]
                nc.vector.tensor_copy(
                    hf_r[:, b],
                    src.rearrange("p (k b) -> p b k", k=2)[:, b - h * HB])

            # ---- e2d projection: instrT = tanh(e2d_w @ h + b) -------------
            instrT = cellpool.tile([P, 2 * B], bf16, tag="instrT")  # (kh, b)
            for m in range(2):
                pe2 = gpsum.tile([P, B], f32, tag="e2d")
                for kh in range(2):
                    nc.tensor.matmul(
                        pe2[:],
                        lhsT=e2dT[:, (kh * 2 + m) * P:(kh * 2 + m + 1) * P],
                        rhs=h_fin[:, kh * B:(kh + 1) * B],
                        start=(kh == 0), stop=(kh == 1),
                    )
                nc.scalar.activation(out=instrT[:, m * B:(m + 1) * B],
                                     in_=pe2[:], func=AFT.Tanh,
                                     bias=e2db[:, m:m + 1])
            lstm_psum.__exit__(None, None, None)

            # ---- lin1 (r-slice): f1T[c, (ct,b,r)] = Lrelu(W_ct_r @ instr + b1)
            f1_sb = f1pool.tile([P, 8 * B * RS], bf16)
            CW = RS * 2 * P  # w1 chunk cols per ct
            lin1_psum = tc.tile_pool(name="lpsum", bufs=4, space="PSUM")
            lpsum = lin1_psum.__enter__()
            for ct in range(8):
                wch = w1pool.tile([P, CW], bf16, tag="w1c")
                nc.sync.dma_start(out=wch[:], in_=w1_d[:, ct * CW:(ct + 1) * CW])
                pb = lpsum.tile([P, RS * B], f32, tag="lin1")
                nc.tensor.matmul(pb[:], lhsT=b1[:, ct * P:(ct + 1) * P],
                                 rhs=delta[:], start=True, stop=False,
                                 skip_group_check=True)
                for r in range(RS):
                    for kh in range(2):
                        nc.tensor.matmul(
                            pb[:, r * B:(r + 1) * B],
                            lhsT=wch[:, (r * 2 + kh) * P:(r * 2 + kh + 1) * P],
                            rhs=instrT[:, kh * B:(kh + 1) * B],
                            start=False, stop=(r == RS - 1 and kh == 1),
                            skip_group_check=True,
                        )
                out_ap = (f1_sb[:, ct * B * RS:(ct + 1) * B * RS]
                          .rearrange("p (b r) -> p r b", b=B))
                nc.scalar.activation(out=out_ap, in_=pb[:], func=AFT.Lrelu,
                                     alpha=0.01)
            lin1_psum.__exit__(None, None, None)

            # ---- conv per sample + fused BN-shift + channel max ----------
            vout = vpool.tile([P, B * 8], f32)  # (b, m)
            conv_psum = tc.tile_pool(name="cpsum", bufs=4, space="PSUM")
            cpsum = conv_psum.__enter__()
            for b in range(B):
                fb = fpool.tile([P, 8 * HW2], bf16, tag="feat")
                nc.sync.dma_start(
                    out=fb[:].rearrange("p (kc hw) -> p kc hw", kc=8),
                    in_=feat_d[b])
                pc = cpsum.tile([P, 8 * RS], f32, tag="conv")
                nc.tensor.matmul(pc[:], lhsT=ones32[:], rhs=trow[:],
                                 start=True, stop=False, skip_group_check=True)
                for m in range(8):
                    for kc in range(8):
                        nc.tensor.matmul(
                            pc[:, m * RS:(m + 1) * RS],
                            lhsT=fb[:, kc * HW2 + m * P: kc * HW2 + (m + 1) * P],
                            rhs=f1_sb[:, kc * B * RS + b * RS:
                                      kc * B * RS + (b + 1) * RS],
                            start=False, stop=(m == 7 and kc == 7),
                            skip_group_check=True,
                        )
                nc.vector.tensor_reduce(
                    out=vout[:, b * 8:(b + 1) * 8],
                    in_=pc[:].rearrange("p (m r) -> p m r", m=8),
                    axis=AX.X, op=AluOpType.max)
            conv_psum.__exit__(None, None, None)

            # contiguous store; host un-permutes (p,(b,m)) -> (b, m*128+p)
            nc.sync.dma_start(out=out_d, in_=vout[:])

    _split_excess_waits(nc)
    return nc


# ---------------------------------------------------------------------------
def _prep_inputs(feature, instruction_idx, instruction_length, emb_table,
                 w_ih, w_hh, b_ih, b_hh, e2d_w, e2d_b,
                 lin1_w, lin1_b, bn_gamma, bn_beta, bn_mean, bn_var):
    """Host-side layout/dtype prep. Returns (in_maps, slots, T0)."""
    f32 = np.float32

    def to_bf(x):
        return np.ascontiguousarray(x.astype(BF16))

    feature = np.asarray(feature, f32)
    emb_table = np.asarray(emb_table, f32)
    idx = np.asarray(instruction_idx)
    lengths = np.asarray(instruction_length).astype(np.int64)
    slots = [int(max(l, 1) - 1) for l in lengths]

    # feature (b, c_in, kc, hw): per-partition data contiguous (16KB) so the
    # DMA uses 16KB descriptors instead of 2KB
    feat = to_bf(feature.reshape(B, 8, P, HW2).transpose(0, 2, 1, 3))

    # embeds transposed: [p, (ke, t*b)]
    emb = emb_table[idx]                       # [B, S, E]
    embT = emb.transpose(2, 1, 0).reshape(2, P, S * B)
    embT = to_bf(embT.transpose(1, 0, 2).reshape(P, 2 * S * B))

    def wtiles(w, kt, mt):
        # w: [out, in] -> lhsT tiles arr[p, (k, m, col)] with lhsT=w.T tile
        wt = np.asarray(w, f32).T  # [in, out]
        a = wt.reshape(kt, P, mt, P).transpose(1, 0, 2, 3)
        return to_bf(a.reshape(P, kt * mt * P))

    # tanh(g) computed as 2*sigmoid(2g)-1: scale the g-gate rows (512:768)
    # by 2 so one big sigmoid covers all four gates.
    gsc = np.ones((4 * HID, 1), f32)
    gsc[2 * HID:3 * HID] = 2.0
    wihT = wtiles(np.asarray(w_ih, f32) * gsc, 2, 8)
    whhT = wtiles(np.asarray(w_hh, f32) * gsc, 2, 8)
    e2dT = wtiles(e2d_w, 2, 2)

    bg = ((np.asarray(b_ih, f32) + np.asarray(b_hh, f32)) * gsc[:, 0]) \
        .reshape(8, P).T.copy()
    e2db = np.asarray(e2d_b, f32).reshape(2, P).T.copy()

    s = np.asarray(bn_gamma, f32) / np.sqrt(np.asarray(bn_var, f32) + BN_EPS)
    tsh = np.asarray(bn_beta, f32) - np.asarray(bn_mean, f32) * s
    T0 = float(tsh.max())

    w1s = np.asarray(lin1_w, f32).reshape(R, C, HID) * s[:, None, None]
    b1s = np.asarray(lin1_b, f32).reshape(R, C) * s[:, None]

    delta = np.repeat(np.eye(RS, dtype=f32), B, axis=1)  # [32, 512]
    eye = np.eye(P, dtype=f32)
    ones32 = np.ones((RS, P), f32)

    def pad128(a):
        out = np.zeros((P, a.shape[1]), f32)
        out[:a.shape[0]] = a
        return out

    biasf = np.concatenate([bg, e2db], axis=1).astype(f32)  # [128, 10]
    biasf = np.ascontiguousarray(biasf)

    in_maps = []
    for k in range(N_CORES):
        rsl = slice(k * RS, (k + 1) * RS)
        wsl = w1s[rsl]                          # [32, 1024, 256] (r, c, h)
        # tiles (ct, r, kh): arr[p, ...] = w.T[kh*128+p, r, ct*128+col]
        ws = wsl.transpose(2, 1, 0)             # [h, c, r]
        a = (ws.reshape(2, P, 8, P, RS)         # [kh, p, ct, col, r]
             .transpose(1, 2, 4, 0, 3)          # [p, ct, r, kh, col]
             .reshape(P, 8 * RS * 2 * P))
        b1c = b1s[rsl].reshape(RS, 8, P).reshape(RS, 8 * P)  # (r, (ct, c))
        tr = np.zeros((RS, 8 * RS), f32)
        tr[0] = np.tile(tsh[rsl], 8)
        cpack = np.concatenate(
            [embT.astype(f32), wihT.astype(f32), whhT.astype(f32),
             e2dT.astype(f32), eye, pad128(b1c), pad128(tr), pad128(delta),
             pad128(ones32)], axis=1)
        in_maps.append(dict(feat=feat, cpack=to_bf(cpack), biasf=biasf,
                            w1T=to_bf(a)))
    return in_maps, slots, T0


_cache = {}


def _run(inputs, trace=False):
    (in_maps, slots, T0) = _prep_inputs(
        inputs["feature"], inputs["instruction_idx"],
        inputs["instruction_length"], inputs["emb_table"],
        inputs["w_ih"], inputs["w_hh"], inputs["b_ih"], inputs["b_hh"],
        inputs["e2d_w"], inputs["e2d_b"], inputs["lin1_w"], inputs["lin1_b"],
        inputs["bn_gamma"], inputs["bn_beta"], inputs["bn_mean"],
        inputs["bn_var"])

    key = tuple(slots)
    if key not in _cache:
        _cache[key] = _build_program(slots)
    nc = _cache[key]

    kw = {}
    if trace:
        kw = dict(trace=True, trace_cores=list(range(N_CORES)))
    res = run_bass_kernel_spmd(nc, in_maps, list(range(N_CORES)), **kw)
    parts = np.stack([np.asarray(res.results[i]["part_out"], np.float32)
                      for i in range(N_CORES)])        # [8, 128(p), 128(b,m)]
    single = parts.max(axis=0)                          # [p, (b, m)]
    # (p, (b, m)) -> (b, m*128+p)
    single = single.reshape(P, B, 8).transpose(1, 2, 0).reshape(B, HW2)
    single = np.maximum(single, T0)
    out = np.clip(single, 0.0, 1.0).reshape(B, 32, 32).astype(np.float32)
    return out, res


def kernel(**inputs) -> np.ndarray:
    out, _ = _run(inputs, trace=False)
    return out


def kernel_traced(**inputs):
    out, res = _run(inputs, trace=True)
    return out, res
